# revision 1
# baseline (speedup 1.0000x reference)
"""GAT 2-layer kernel for 8 Trainium2 NeuronCores (SPMD via bass/Tile).

Strategy:
  - dst-shard nodes across 8 cores; edges grouped by owner core, then by
    128-wide local dst block, sorted by src within a block.
  - Per layer a DRAM "table" holds one row per node:
      [128 x bf16 feat | el0 el1 er0 er1 f32] = 68 f32 cols (272B).
    fc is data-parallel over node shards; AllGather replicates the table.
  - Edge phase: per dst block, gather 128 edge rows per indirect DMA,
    compute ex = exp(leakyrelu(el[src]+er[dst])), scale gathered feats,
    aggregate with a one-hot matmul (P^T @ [G*ex | ex]) accumulating in
    PSUM; normalize by the z columns afterwards.
  - Attention logits el/er are folded into the fc matmul via W@a products.
"""

import sys

import numpy as np

sys.path.insert(0, "/opt/trn_rl_repo")

import ml_dtypes

# problem sizes (overridable via configure() for scaled-down testing)
N = 50000
E = 800000
IN, HID, OUT = 128, 64, 64
H = 2
NEG = 0.2
NCORES = 8
P = 128
NSHARD = N // NCORES
NBLK = (NSHARD + P - 1) // P
RSLICE = NBLK * P + P         # table rows per rank slice (1 pad + nodes + junk)
TROWS = RSLICE * NCORES
TCOLS = 68
PAD_EL = -100.0


def configure(n, e):
    """Adjust module globals for a scaled-down test configuration."""
    global N, E, NSHARD, NBLK, RSLICE, TROWS
    N, E = n, e
    NSHARD = N // NCORES
    NBLK = (NSHARD + P - 1) // P
    RSLICE = NBLK * P + P
    TROWS = RSLICE * NCORES


def _row_of(node):
    return RSLICE * (node // NSHARD) + 1 + (node % NSHARD)


def preprocess(a):
    bf16 = ml_dtypes.bfloat16
    f32 = np.float32
    h, src, dst = a["h"], a["src"].astype(np.int64), a["dst"].astype(np.int64)

    def rhs_pack(W, al, ar):
        k = W.shape[0]
        Wr = W.reshape(k, H, W.shape[1] // H)
        wal = np.einsum("ihj,hj->ih", Wr, al)
        war = np.einsum("ihj,hj->ih", Wr, ar)
        return np.concatenate([W, wal, war], axis=1).astype(bf16)

    rhsW1 = rhs_pack(a["W1"].astype(f32), a["al1"].astype(f32), a["ar1"].astype(f32))
    rhsW2 = rhs_pack(a["W2"].astype(f32), a["al2"].astype(f32), a["ar2"].astype(f32))
    lin1_bp = a["lin1_b"].astype(f32) + a["b1"].astype(f32) @ a["lin1_W"].astype(f32)
    lin2_bp = a["lin2_b"].astype(f32) + a["b2"].astype(f32) @ a["lin2_W"].astype(f32)

    core = dst // NSHARD
    local = dst - core * NSHARD
    blk = local // P
    dloc = local % P
    order = np.lexsort((src, blk, core))
    src_s, core_s, blk_s, dloc_s = src[order], core[order], blk[order], dloc[order]

    counts = np.zeros((NCORES, NBLK), np.int64)
    np.add.at(counts, (core_s, blk_s), 1)
    sb = np.maximum(1, (counts.max(axis=0) + P - 1) // P)
    tiles_total = int(sb.sum())
    slots_total = tiles_total * P

    idx_arr = np.zeros((NCORES, slots_total), np.int32)
    dloc_arr = np.zeros((NCORES, slots_total), np.float32)
    tile_base = np.concatenate([[0], np.cumsum(sb)]).astype(int)
    core_cnt = np.bincount(core_s, minlength=NCORES)
    core_off = np.concatenate([[0], np.cumsum(core_cnt)]).astype(int)
    rows_all = _row_of(src_s).astype(np.int32)
    for c in range(NCORES):
        base = core_off[c]
        cnt = np.bincount(blk_s[core_s == c], minlength=NBLK)
        st = np.concatenate([[0], np.cumsum(cnt)]).astype(int)
        for b in range(NBLK):
            e0, e1 = st[b], st[b + 1]
            s0 = tile_base[b] * P
            idx_arr[c, s0:s0 + (e1 - e0)] = rows_all[base + e0:base + e1]
            dloc_arr[c, s0:s0 + (e1 - e0)] = dloc_s[base + e0:base + e1]
    idx_pt = np.ascontiguousarray(
        idx_arr.reshape(NCORES, tiles_total, P).transpose(0, 2, 1))
    dloc_pt = np.ascontiguousarray(
        dloc_arr.reshape(NCORES, tiles_total, P).transpose(0, 2, 1)
        ).astype(bf16)

    NCOLS = NBLK * P
    hT = np.zeros((NCORES, IN, NCOLS), bf16)
    for c in range(NCORES):
        hT[c, :, :NSHARD] = h[c * NSHARD:(c + 1) * NSHARD].T.astype(bf16)

    padrow = np.zeros((1, TCOLS), f32)
    padrow[0, 64] = PAD_EL
    padrow[0, 65] = PAD_EL
    iota2d = np.tile(np.arange(P, dtype=f32), (P, 1)).astype(bf16)

    shared = dict(rhsW1=rhsW1, rhsW2=rhsW2,
                  lin1W=a["lin1_W"].astype(bf16), lin2W=a["lin2_W"].astype(bf16),
                  b1col=lin1_bp.reshape(HID, 1).astype(f32),
                  b2row=np.tile(lin2_bp[None, :], (P, 1)).astype(f32),
                  padrow=padrow, iota2d=iota2d,
                  iotaP=np.arange(P, dtype=f32).reshape(P, 1).astype(bf16))
    per_core = [dict(hT=np.ascontiguousarray(hT[c]), idx=idx_pt[c],
                     dloc=dloc_pt[c],
                     dloc_flat=dloc_arr[c].reshape(1, -1).astype(bf16))
                for c in range(NCORES)]
    return shared, per_core, sb


def build(sb, repeat=1, variant="all"):
    import concourse.bass as bass
    import concourse.bacc as bacc
    import concourse.mybir as mybir
    from concourse import tile

    dt = mybir.dt
    NBLKS = NBLK
    tiles_total = int(sum(sb))
    tile_base = np.concatenate([[0], np.cumsum(sb)]).astype(int)
    NCOLS = NBLK * P

    nc = bacc.Bacc("TRN2", target_bir_lowering=False, debug=False,
                   num_devices=NCORES)

    def din(name, shape, dty):
        return nc.dram_tensor(name, shape, dty, kind="ExternalInput").ap()

    d_hT = din("hT", [IN, NCOLS], dt.bfloat16)
    d_idx = din("idx", [P, tiles_total], dt.int32)
    d_dloc = din("dloc", [P, tiles_total], dt.bfloat16)
    d_rhsW1 = din("rhsW1", [IN, 132], dt.bfloat16)
    d_rhsW2 = din("rhsW2", [HID, 132], dt.bfloat16)
    d_lin1W = din("lin1W", [2 * HID, HID], dt.bfloat16)
    d_lin2W = din("lin2W", [2 * OUT, OUT], dt.bfloat16)
    d_b1col = din("b1col", [HID, 1], dt.float32)
    d_b2row = din("b2row", [P, OUT], dt.float32)
    d_padrow = din("padrow", [1, TCOLS], dt.float32)
    d_iota = din("iota2d", [P, P], dt.bfloat16)
    d_iotaP = din("iotaP", [P, 1], dt.bfloat16)
    d_dlocflat = din("dloc_flat", [1, tiles_total * P], dt.bfloat16)
    d_out = nc.dram_tensor("out", [NSHARD, OUT], dt.float32,
                           kind="ExternalOutput").ap()

    with tile.TileContext(nc) as tc:
        with (
            tc.tile_pool(name="const", bufs=1) as cpool,
            tc.tile_pool(name="dram", bufs=1, space="DRAM") as dpool,
            tc.tile_pool(name="big", bufs=1) as bigpool,
        ):
            idx_t = cpool.tile([P, tiles_total], dt.int32)
            dloc_t = cpool.tile([P, tiles_total], dt.bfloat16)
            iota_t = cpool.tile([P, P], dt.bfloat16)
            iotaP_t = cpool.tile([P, 1], dt.bfloat16)
            b2_t = cpool.tile([P, OUT], dt.float32)
            b1_t = cpool.tile([HID, 1], dt.float32)
            pad_t = cpool.tile([1, TCOLS], dt.float32)
            rhs1_t = cpool.tile([IN, 132], dt.bfloat16)
            rhs2_t = cpool.tile([HID, 132], dt.bfloat16)
            l1w_t = cpool.tile([2 * HID, HID], dt.bfloat16)
            l2w_t = cpool.tile([2 * OUT, OUT], dt.bfloat16)
            for t, d in ((idx_t, d_idx), (iotaP_t, d_iotaP),
                         (dloc_t, d_dloc), (iota_t, d_iota),
                         (b2_t, d_b2row), (b1_t, d_b1col), (pad_t, d_padrow),
                         (rhs1_t, d_rhsW1), (rhs2_t, d_rhsW2),
                         (l1w_t, d_lin1W), (l2w_t, d_lin2W)):
                nc.sync.dma_start(t[:], d[:])

            hT_t = bigpool.tile([IN, NCOLS], dt.bfloat16)
            nc.sync.dma_start(hT_t[:], d_hT[:])


            er1_sb = bigpool.tile([P, NBLKS, H], dt.bfloat16)
            er2_sb = bigpool.tile([P, NBLKS, H], dt.bfloat16)
            t1 = bigpool.tile([P, NBLKS, P], dt.bfloat16)
            t1T = bigpool.tile([P, NCOLS], dt.bfloat16)
            xT = bigpool.tile([HID, NCOLS], dt.bfloat16)
            t2 = bigpool.tile([P, NBLKS, P], dt.bfloat16)
            t2T = bigpool.tile([P, NCOLS], dt.bfloat16)

            def fc_phase(lhs_tile, rhs_t, slice_d, er_sb, pool_fc, pool_stg):
                nc.sync.dma_start(slice_d[0:1, :], pad_t[:])
                for nb in range(NBLKS):
                    ps = pool_fc.tile([P, 132], dt.float32, tag="fcps")
                    nc.tensor.matmul(ps[:], lhs_tile[:, nb * P:(nb + 1) * P],
                                     rhs_t[:], start=True, stop=True)
                    stg = pool_stg.tile([P, TCOLS], dt.float32, tag="fcstg")
                    nc.scalar.activation(
                        stg[:].bitcast(dt.bfloat16)[:, 0:128], ps[:, 0:128],
                        mybir.ActivationFunctionType.Copy)
                    nc.vector.tensor_copy(stg[:, 64:68], ps[:, 128:132])
                    nc.vector.tensor_copy(er_sb[:, nb, :], ps[:, 130:132])
                    nc.scalar.dma_start(
                        slice_d[1 + nb * P:1 + (nb + 1) * P, :], stg[:])

            def edge_phase(tab_d, t_stg, er_sb, pool_g, pool_e, pool_ps):
                for b in range(NBLKS):
                    S = int(sb[b])
                    t0 = int(tile_base[b])
                    G = pool_g.tile([P, S, TCOLS], dt.float32, tag="g")
                    for s in range(S):
                        nc.gpsimd.indirect_dma_start(
                            out=G[:, s, :], out_offset=None, in_=tab_d[:],
                            in_offset=bass.IndirectOffsetOnAxis(
                                ap=idx_t[:, t0 + s:t0 + s + 1], axis=0))
                    dlocF = pool_e.tile([P, S * P], dt.bfloat16, tag="dlocF")
                    nc.scalar.dma_start(
                        dlocF[:],
                        d_dlocflat[0:1, t0 * P:(t0 + S) * P]
                        .partition_broadcast(P).squeeze(1))
                    PmT = pool_e.tile([P, S, P], dt.bfloat16, tag="pmt")
                    nc.vector.tensor_tensor(
                        PmT[:],
                        iotaP_t[:].unsqueeze(1).broadcast_to([P, S, P]),
                        dlocF[:].rearrange("p (s e) -> p s e", s=S),
                        mybir.AluOpType.is_equal)
                    er_ps = pool_ps.tile([P, H * S], dt.float32, tag="erps")
                    for s in range(S):
                        nc.tensor.matmul(er_ps[:, H * s:H * (s + 1)],
                                         PmT[:, s, :], er_sb[:, b, :],
                                         start=True, stop=True)
                    ex = pool_e.tile([P, S, H], dt.float32, tag="ex")
                    tmp = pool_e.tile([P, S, H], dt.float32, tag="tmp")
                    nc.vector.tensor_tensor(
                        ex[:], G[:, :, 64:66],
                        er_ps[:].rearrange("p (s h) -> p s h", s=S),
                        mybir.AluOpType.add)
                    nc.vector.tensor_scalar(tmp[:], ex[:], NEG, None,
                                            mybir.AluOpType.mult)
                    nc.vector.tensor_tensor(ex[:], ex[:], tmp[:],
                                            mybir.AluOpType.max)
                    stg = pool_e.tile([P, S, 130], dt.bfloat16, tag="stg")
                    nc.scalar.activation(stg[:, :, 128:130], ex[:],
                                         mybir.ActivationFunctionType.Exp)
                    Gb = G[:].bitcast(dt.bfloat16)
                    for hh in range(H):
                        nc.vector.tensor_tensor(
                            stg[:, :, hh * 64:(hh + 1) * 64],
                            Gb[:, :, hh * 64:(hh + 1) * 64],
                            stg[:, :, 128 + hh:129 + hh].broadcast_to([P, S, 64]),
                            mybir.AluOpType.mult)
                    Pm = pool_e.tile([P, S, P], dt.bfloat16, tag="pm")
                    nc.vector.tensor_tensor(
                        Pm[:],
                        dloc_t[:, t0:t0 + S].unsqueeze(-1).broadcast_to([P, S, P]),
                        iota_t[:].unsqueeze(1).broadcast_to([P, S, P]),
                        mybir.AluOpType.is_equal)
                    acc = pool_ps.tile([P, 130], dt.float32, tag="acc")
                    for s in range(S):
                        nc.tensor.matmul(acc[:], Pm[:, s, :], stg[:, s, :],
                                         start=(s == 0), stop=(s == S - 1))
                    rz = pool_e.tile([P, H], dt.float32, tag="rz")
                    nc.vector.tensor_scalar(rz[:], acc[:, 128:130], 1e-30, None,
                                            mybir.AluOpType.add)
                    nc.vector.reciprocal(rz[:], rz[:])
                    for hh in range(H):
                        nc.vector.tensor_scalar(
                            t_stg[:, b, hh * 64:(hh + 1) * 64],
                            acc[:, hh * 64:(hh + 1) * 64],
                            rz[:, hh:hh + 1], None, mybir.AluOpType.mult)

            def gather_only(tab_d, pool_g, rep):
                for b in range(NBLKS):
                    S = int(sb[b])
                    t0 = int(tile_base[b])
                    G = pool_g.tile([P, S, TCOLS], dt.float32, tag="g")
                    for s in range(S):
                        nc.gpsimd.indirect_dma_start(
                            out=G[:, s, :], out_offset=None, in_=tab_d[:],
                            in_offset=bass.IndirectOffsetOnAxis(
                                ap=idx_t[:, t0 + s:t0 + s + 1], axis=0))
                    jk = pool_g.tile([P, 1], dt.float32, tag="jk")
                    nc.vector.tensor_copy(jk[:], G[:, 0, 0:1])

            if variant in ("edge1", "gath1"):
                slice1 = dpool.tile([RSLICE, TCOLS], dt.float32)
                tab1 = dpool.tile([TROWS, TCOLS], dt.float32,
                                  addr_space="Shared")
                with (
                    tc.tile_pool(name="fcps", bufs=2, space="PSUM") as fcps,
                    tc.tile_pool(name="fcstg", bufs=3) as fcstg,
                ):
                    fc_phase(hT_t, rhs1_t, slice1, er1_sb, fcps, fcstg)
                nc.gpsimd.collective_compute(
                    "AllGather", mybir.AluOpType.bypass,
                    replica_groups=[list(range(NCORES))],
                    ins=[slice1.opt()], outs=[tab1.opt()])
                for _rep in range(repeat):
                    with (
                        tc.tile_pool(name="gpool", bufs=4) as gpool,
                        tc.tile_pool(name="epool", bufs=4) as epool,
                        tc.tile_pool(name="pspool", bufs=4, space="PSUM") as psp,
                    ):
                        if variant == "edge1":
                            edge_phase(tab1, t1, er1_sb, gpool, epool, psp)
                        else:
                            gather_only(tab1, gpool, _rep)
                repeat = 0  # skip the full pipeline below

            for _rep in range(repeat):
                slice1 = dpool.tile([RSLICE, TCOLS], dt.float32,
                                    tag=f"slice1r{_rep}")
                tab1 = dpool.tile([TROWS, TCOLS], dt.float32,
                                  addr_space="Shared", tag=f"tab1r{_rep}")
                slice2 = dpool.tile([RSLICE, TCOLS], dt.float32,
                                    tag=f"slice2r{_rep}")
                tab2 = dpool.tile([TROWS, TCOLS], dt.float32,
                                  addr_space="Shared", tag=f"tab2r{_rep}")
                with (
                    tc.tile_pool(name="fcps", bufs=2, space="PSUM") as fcps,
                    tc.tile_pool(name="fcstg", bufs=3) as fcstg,
                ):
                    fc_phase(hT_t, rhs1_t, slice1, er1_sb, fcps, fcstg)
                nc.gpsimd.collective_compute(
                    "AllGather", mybir.AluOpType.bypass,
                    replica_groups=[list(range(NCORES))],
                    ins=[slice1.opt()], outs=[tab1.opt()])

                with (
                    tc.tile_pool(name="gpool", bufs=4) as gpool,
                    tc.tile_pool(name="epool", bufs=4) as epool,
                    tc.tile_pool(name="pspool", bufs=4, space="PSUM") as pspool,
                ):
                    edge_phase(tab1, t1, er1_sb, gpool, epool, pspool)

                for nb in range(NBLKS):
                    nc.sync.dma_start_transpose(t1T[:, nb * P:(nb + 1) * P],
                                                t1[:, nb, :])
                with (
                    tc.tile_pool(name="x2ps", bufs=2, space="PSUM") as x2ps,
                    tc.tile_pool(name="fc2stg", bufs=3) as fc2stg,
                ):
                    CH = 512
                    nch = NCOLS // CH
                    rem = NCOLS - nch * CH
                    for ch in range(nch + (1 if rem else 0)):
                        w = CH if ch < nch else rem
                        ps = x2ps.tile([HID, CH], dt.float32, tag="xps")
                        nc.tensor.matmul(ps[:, 0:w], l1w_t[:],
                                         t1T[:, ch * CH:ch * CH + w],
                                         start=True, stop=True)
                        nc.scalar.activation(xT[:, ch * CH:ch * CH + w],
                                             ps[:, 0:w],
                                             mybir.ActivationFunctionType.Relu,
                                             bias=b1_t[:])
                    fc_phase(xT, rhs2_t, slice2, er2_sb, x2ps, fc2stg)
                nc.gpsimd.collective_compute(
                    "AllGather", mybir.AluOpType.bypass,
                    replica_groups=[list(range(NCORES))],
                    ins=[slice2.opt()], outs=[tab2.opt()])

                with (
                    tc.tile_pool(name="gpool2", bufs=4) as gpool2,
                    tc.tile_pool(name="epool2", bufs=4) as epool2,
                    tc.tile_pool(name="pspool2", bufs=4, space="PSUM") as pspool2,
                ):
                    edge_phase(tab2, t2, er2_sb, gpool2, epool2, pspool2)

                for nb in range(NBLKS):
                    nc.sync.dma_start_transpose(t2T[:, nb * P:(nb + 1) * P],
                                                t2[:, nb, :])
                with (
                    tc.tile_pool(name="ops", bufs=2, space="PSUM") as ops,
                    tc.tile_pool(name="ostg", bufs=3) as ostg,
                ):
                    for nb in range(NBLKS):
                        ps = ops.tile([P, OUT], dt.float32, tag="ops")
                        nc.tensor.matmul(ps[:], t2T[:, nb * P:(nb + 1) * P],
                                         l2w_t[:], start=True, stop=True)
                        og = ostg.tile([P, OUT], dt.float32, tag="og")
                        nc.vector.tensor_tensor(og[:], ps[:], b2_t[:],
                                                mybir.AluOpType.add)
                        r0 = nb * P
                        r1 = min(r0 + P, NSHARD)
                        if r1 > r0:
                            nc.sync.dma_start(d_out[r0:r1, :], og[0:r1 - r0, :])

    nc.compile()
    return nc


def kernel(**inputs) -> np.ndarray:
    from concourse.bass_utils import run_bass_kernel_spmd

    args = {k: np.asarray(v) for k, v in inputs.items()}
    shared, per_core, sb = preprocess(args)
    nc = build(sb)
    in_maps = [{**shared, **pc} for pc in per_core]
    res = run_bass_kernel_spmd(nc, in_maps, list(range(NCORES)))
    out = np.concatenate([res.results[c]["out"] for c in range(NCORES)], axis=0)
    return np.ascontiguousarray(out.astype(np.float32))



# revision 6
# speedup vs baseline: 44.8254x; 44.8254x over previous
"""GAT 2-layer kernel for 8 Trainium2 NeuronCores (SPMD via bass/Tile).

Strategy:
  - dst-shard nodes across 8 cores; edges grouped by owner core, then by
    128-wide local dst block, then by src-table half, sorted by src.
  - Per layer a DRAM "table" holds one 512B row per node:
      [128 x bf16 feat | el0 el1 er0 er1 f32 | pad] = 128 f32 cols.
    fc is data-parallel over node shards; AllGather replicates the table.
  - Edge phase: per dst block, TWO batched dma_gather ops (one per table
    half, int16 local indices) fetch all edge rows at once; compute
    ex = exp(leakyrelu(el[src]+er[dst])), scale gathered feats, aggregate
    with one-hot matmuls (Pm^T @ [G*ex | ex]) accumulating in PSUM;
    normalize by the z columns afterwards.
  - er[dst] dispersal per edge via one-hot PmT matmuls built cheaply with
    a 4x-mode tensor_scalar; DVE ops are pair-packed for 2x mode.
  - Attention logits el/er are folded into the fc matmul via W@a products.
"""

import sys

import numpy as np

sys.path.insert(0, "/opt/trn_rl_repo")

import ml_dtypes

# problem sizes (overridable via configure() for scaled-down testing)
N = 50000
E = 800000
IN, HID, OUT = 128, 64, 64
H = 2
NEG = 0.2
NCORES = 8
P = 128
NSHARD = N // NCORES
NBLK = (NSHARD + P - 1) // P
RSLICE = NBLK * P + P         # table rows per rank slice (1 pad + nodes + junk)
TROWS = RSLICE * NCORES
HROWS = TROWS // 2            # rows per gather half (must fit int16)
TCOLS = 128                   # table row: 512B
PAD_EL = -100.0


def configure(n, e):
    """Adjust module globals for a scaled-down test configuration."""
    global N, E, NSHARD, NBLK, RSLICE, TROWS, HROWS
    N, E = n, e
    NSHARD = N // NCORES
    NBLK = (NSHARD + P - 1) // P
    RSLICE = NBLK * P + P
    TROWS = RSLICE * NCORES
    HROWS = TROWS // 2


def _row_of(node):
    return RSLICE * (node // NSHARD) + 1 + (node % NSHARD)


def preprocess(a):
    bf16 = ml_dtypes.bfloat16
    f32 = np.float32
    h, src, dst = a["h"], a["src"].astype(np.int64), a["dst"].astype(np.int64)
    assert HROWS <= 32768

    def rhs_pack(W, al, ar):
        k = W.shape[0]
        Wr = W.reshape(k, H, W.shape[1] // H)
        wal = np.einsum("ihj,hj->ih", Wr, al)
        war = np.einsum("ihj,hj->ih", Wr, ar)
        return np.concatenate([W, wal, war], axis=1).astype(bf16)

    rhsW1 = rhs_pack(a["W1"].astype(f32), a["al1"].astype(f32), a["ar1"].astype(f32))
    rhsW2 = rhs_pack(a["W2"].astype(f32), a["al2"].astype(f32), a["ar2"].astype(f32))
    lin1_bp = a["lin1_b"].astype(f32) + a["b1"].astype(f32) @ a["lin1_W"].astype(f32)
    lin2_bp = a["lin2_b"].astype(f32) + a["b2"].astype(f32) @ a["lin2_W"].astype(f32)

    core = dst // NSHARD
    local = dst - core * NSHARD
    blk = local // P
    dloc = local % P
    grow = _row_of(src)
    half = grow // HROWS
    lrow = grow - half * HROWS
    order = np.lexsort((src, half, blk, core))
    core_s, blk_s, half_s = core[order], blk[order], half[order]
    dloc_s, lrow_s = dloc[order], lrow[order]

    counts = np.zeros((NCORES, NBLK, 2), np.int64)
    np.add.at(counts, (core_s, blk_s, half_s), 1)
    sbh = -(-counts.max(axis=0) // P)          # [NBLK, 2] tiles per (blk, half)
    for b in range(NBLK):
        if sbh[b].sum() == 0:
            sbh[b, 0] = 1
    tiles_total = int(sbh.sum())
    slots_total = tiles_total * P
    tile_base = np.zeros((NBLK, 2), np.int64)  # global tile index of (b, h)
    acc = 0
    for b in range(NBLK):
        for hh in range(2):
            tile_base[b, hh] = acc
            acc += sbh[b, hh]

    idx_arr = np.zeros((NCORES, slots_total), np.int16)
    dloc_arr = np.zeros((NCORES, slots_total), np.float32)
    for c in range(NCORES):
        m = core_s == c
        cb, ch = blk_s[m], half_s[m]
        cd, cl = dloc_s[m], lrow_s[m]
        cnt = counts[c]                        # [NBLK, 2]
        off = 0
        for b in range(NBLK):
            for hh in range(2):
                n_ = int(cnt[b, hh])
                if n_:
                    s0 = int(tile_base[b, hh]) * P
                    idx_arr[c, s0:s0 + n_] = cl[off:off + n_]
                    dloc_arr[c, s0:s0 + n_] = cd[off:off + n_]
                    off += n_
    # wrapped int16 index layout: index j -> [j%16 (+16r), j//16]
    w16 = idx_arr.reshape(NCORES, slots_total // 16, 16).transpose(0, 2, 1)
    idx16 = np.tile(w16, (1, 8, 1)).astype(np.int16)
    dloc_pt = np.ascontiguousarray(
        dloc_arr.reshape(NCORES, tiles_total, P).transpose(0, 2, 1)).astype(bf16)
    dloc2_pt = np.ascontiguousarray(np.repeat(dloc_pt, 2, axis=2))

    NCOLS = NBLK * P
    hT = np.zeros((NCORES, IN, NCOLS), bf16)
    for c in range(NCORES):
        hT[c, :, :NSHARD] = h[c * NSHARD:(c + 1) * NSHARD].T.astype(bf16)

    padrow = np.zeros((1, TCOLS), f32)
    padrow[0, 64] = PAD_EL
    padrow[0, 65] = PAD_EL
    iota2d = np.tile(np.arange(P, dtype=f32), (P, 1)).astype(bf16)

    shared = dict(rhsW1=rhsW1, rhsW2=rhsW2,
                  lin1W=a["lin1_W"].astype(bf16), lin2W=a["lin2_W"].astype(bf16),
                  b1col=lin1_bp.reshape(HID, 1).astype(f32),
                  b2row=np.tile(lin2_bp[None, :], (P, 1)).astype(f32),
                  padrow=padrow, iota2d=iota2d,
                  iotaP=np.arange(P, dtype=f32).reshape(P, 1))
    per_core = [dict(hT=np.ascontiguousarray(hT[c]), idx16=idx16[c],
                     dloc2=dloc2_pt[c],
                     dloc_flat=dloc_arr[c].reshape(1, -1).astype(bf16))
                for c in range(NCORES)]
    return shared, per_core, sbh


def build(sbh, repeat=1, variant="all"):
    import concourse.bass as bass
    import concourse.bacc as bacc
    import concourse.mybir as mybir
    from concourse import tile

    dt = mybir.dt
    NBLKS = NBLK
    sbh = np.asarray(sbh)
    tiles_total = int(sbh.sum())
    slots_total = tiles_total * P
    tile_base = np.zeros((NBLK, 2), np.int64)
    acc = 0
    for b in range(NBLK):
        for hh in range(2):
            tile_base[b, hh] = acc
            acc += sbh[b, hh]
    NCOLS = NBLK * P

    nc = bacc.Bacc("TRN2", target_bir_lowering=False, debug=False,
                   num_devices=NCORES)

    def din(name, shape, dty):
        return nc.dram_tensor(name, shape, dty, kind="ExternalInput").ap()

    d_hT = din("hT", [IN, NCOLS], dt.bfloat16)
    d_idx16 = din("idx16", [P, slots_total // 16], dt.int16)
    d_dloc2 = din("dloc2", [P, 2 * tiles_total], dt.bfloat16)
    d_rhsW1 = din("rhsW1", [IN, 132], dt.bfloat16)
    d_rhsW2 = din("rhsW2", [HID, 132], dt.bfloat16)
    d_lin1W = din("lin1W", [2 * HID, HID], dt.bfloat16)
    d_lin2W = din("lin2W", [2 * OUT, OUT], dt.bfloat16)
    d_b1col = din("b1col", [HID, 1], dt.float32)
    d_b2row = din("b2row", [P, OUT], dt.float32)
    d_padrow = din("padrow", [1, TCOLS], dt.float32)
    d_iota = din("iota2d", [P, P], dt.bfloat16)
    d_iotaP = din("iotaP", [P, 1], dt.float32)
    d_dlocflat = din("dloc_flat", [1, slots_total], dt.bfloat16)
    d_out = nc.dram_tensor("out", [NSHARD, OUT], dt.float32,
                           kind="ExternalOutput").ap()

    with tile.TileContext(nc) as tc:
        with (
            tc.tile_pool(name="const", bufs=1) as cpool,
            tc.tile_pool(name="dram", bufs=1, space="DRAM") as dpool,
            tc.tile_pool(name="big", bufs=1) as bigpool,
        ):
            idx16_t = cpool.tile([P, slots_total // 16], dt.int16)
            dloc2_t = cpool.tile([P, 2 * tiles_total], dt.bfloat16)
            iota_t = cpool.tile([P, P], dt.bfloat16)
            iotaP_t = cpool.tile([P, 1], dt.float32)
            b2_t = cpool.tile([P, OUT], dt.float32)
            b1_t = cpool.tile([HID, 1], dt.float32)
            pad_t = cpool.tile([1, TCOLS], dt.float32)
            rhs1_t = cpool.tile([IN, 132], dt.bfloat16)
            rhs2_t = cpool.tile([HID, 132], dt.bfloat16)
            l1w_t = cpool.tile([2 * HID, HID], dt.bfloat16)
            l2w_t = cpool.tile([2 * OUT, OUT], dt.bfloat16)
            for t, d in ((idx16_t, d_idx16), (dloc2_t, d_dloc2),
                         (iota_t, d_iota), (iotaP_t, d_iotaP),
                         (b2_t, d_b2row), (b1_t, d_b1col), (pad_t, d_padrow),
                         (rhs1_t, d_rhsW1), (rhs2_t, d_rhsW2),
                         (l1w_t, d_lin1W), (l2w_t, d_lin2W)):
                nc.sync.dma_start(t[:], d[:])

            hT_t = bigpool.tile([IN, NCOLS], dt.bfloat16)
            nc.sync.dma_start(hT_t[:], d_hT[:])

            er1_sb = bigpool.tile([P, NBLKS, H], dt.bfloat16)
            er2_sb = bigpool.tile([P, NBLKS, H], dt.bfloat16)
            t1 = bigpool.tile([P, NBLKS, P], dt.bfloat16)
            t1T = bigpool.tile([P, NCOLS], dt.bfloat16)
            xT = bigpool.tile([HID, NCOLS], dt.bfloat16)
            t2 = bigpool.tile([P, NBLKS, P], dt.bfloat16)
            t2T = bigpool.tile([P, NCOLS], dt.bfloat16)

            def fc_phase(lhs_tile, rhs_t, slice_d, er_sb, pool_fc, pool_stg):
                nc.sync.dma_start(slice_d[0:1, :], pad_t[:])
                for nb in range(NBLKS):
                    ps = pool_fc.tile([P, 132], dt.float32, tag="fcps")
                    nc.tensor.matmul(ps[:], lhs_tile[:, nb * P:(nb + 1) * P],
                                     rhs_t[:], start=True, stop=True)
                    stg = pool_stg.tile([P, TCOLS], dt.float32, tag="fcstg")
                    nc.scalar.activation(
                        stg[:].bitcast(dt.bfloat16)[:, 0:128], ps[:, 0:128],
                        mybir.ActivationFunctionType.Copy)
                    nc.vector.tensor_copy(stg[:, 64:68], ps[:, 128:132])
                    nc.vector.tensor_copy(er_sb[:, nb, :], ps[:, 130:132])
                    nc.sync.dma_start(
                        slice_d[1 + nb * P:1 + (nb + 1) * P, :], stg[:])

            def edge_phase(tab_d, t_stg, er_sb, pool_g, pool_e, pool_ps,
                           pool_eps):
                for b in range(NBLKS):
                    S0, S1 = int(sbh[b, 0]), int(sbh[b, 1])
                    S = S0 + S1
                    t0 = int(tile_base[b, 0])
                    G = pool_g.tile([P, S, TCOLS], dt.float32, tag="g")
                    for hh, (sh, toff) in enumerate(((S0, 0), (S1, S0))):
                        if sh == 0:
                            continue
                        s0 = (t0 + toff) * P
                        nc.gpsimd.dma_gather(
                            G[:, toff:toff + sh, :],
                            tab_d[hh * HROWS:(hh + 1) * HROWS, :],
                            idx16_t[:, s0 // 16:(s0 + sh * P) // 16],
                            sh * P, sh * P, TCOLS, single_packet=False)
                    dlocF = pool_e.tile([P, S * P], dt.bfloat16, tag="dlocF")
                    nc.sync.dma_start(
                        dlocF[:],
                        d_dlocflat[0:1, t0 * P:(t0 + S) * P]
                        .partition_broadcast(P).squeeze(1))
                    PmT = pool_e.tile([P, S, P], dt.bfloat16, tag="pmt")
                    nc.vector.tensor_scalar(
                        PmT[:].rearrange("p s e -> p (s e)"), dlocF[:],
                        iotaP_t[:, 0:1], None, mybir.AluOpType.is_equal)
                    er_ps = pool_eps.tile([P, H * S], dt.float32, tag="erps")
                    for s in range(S):
                        nc.tensor.matmul(er_ps[:, H * s:H * (s + 1)],
                                         PmT[:, s, :], er_sb[:, b, :],
                                         start=True, stop=True)
                    ex = pool_e.tile([P, S, H], dt.float32, tag="ex")
                    tmp = pool_e.tile([P, S, H], dt.float32, tag="tmp")
                    nc.vector.tensor_tensor(
                        ex[:], G[:, :, 64:66],
                        er_ps[:].rearrange("p (s h) -> p s h", s=S),
                        mybir.AluOpType.add)
                    nc.vector.tensor_scalar(tmp[:], ex[:], NEG, None,
                                            mybir.AluOpType.mult)
                    nc.vector.tensor_tensor(ex[:], ex[:], tmp[:],
                                            mybir.AluOpType.max)
                    stg = pool_e.tile([P, S, 132], dt.bfloat16, tag="stg")
                    exd = stg[:, :, 128:132].rearrange(
                        "p s (h two) -> p s h two", h=H)
                    for k in range(2):
                        nc.scalar.activation(exd[:, :, :, k], ex[:],
                                             mybir.ActivationFunctionType.Exp)
                    Gb = G[:].bitcast(dt.bfloat16)
                    for hh in range(H):
                        nc.vector.tensor_tensor(
                            stg[:, :, hh * 64:(hh + 1) * 64]
                            .rearrange("p s (c two) -> p s c two", two=2),
                            Gb[:, :, hh * 64:(hh + 1) * 64]
                            .rearrange("p s (c two) -> p s c two", two=2),
                            stg[:, :, 128 + 2 * hh:130 + 2 * hh]
                            .rearrange("p s (c two) -> p s c two", two=2)
                            .broadcast_to([P, S, 32, 2]),
                            mybir.AluOpType.mult)
                    Pm = pool_e.tile([P, S, P], dt.bfloat16, tag="pm")
                    nc.vector.tensor_tensor(
                        Pm[:].rearrange("p s (d two) -> p s d two", two=2),
                        dloc2_t[:, 2 * t0:2 * (t0 + S)]
                        .rearrange("p (s two) -> p s two", two=2)
                        .unsqueeze(2).broadcast_to([P, S, 64, 2]),
                        iota_t[:].rearrange("p (d two) -> p d two", two=2)
                        .unsqueeze(1).broadcast_to([P, S, 64, 2]),
                        mybir.AluOpType.is_equal)
                    acc = pool_ps.tile([P, 132], dt.float32, tag="acc")
                    for s in range(S):
                        nc.tensor.matmul(acc[:], Pm[:, s, :], stg[:, s, :],
                                         start=(s == 0), stop=(s == S - 1))
                    rz = pool_e.tile([P, H], dt.float32, tag="rz")
                    nc.vector.tensor_scalar(
                        rz[:],
                        acc[:, 128:132].rearrange("p (h two) -> p h two", h=H)
                        [:, :, 0:1].squeeze(2),
                        1e-30, None, mybir.AluOpType.add)
                    nc.vector.reciprocal(rz[:], rz[:])
                    for hh in range(H):
                        nc.vector.tensor_scalar(
                            t_stg[:, b, hh * 64:(hh + 1) * 64],
                            acc[:, hh * 64:(hh + 1) * 64],
                            rz[:, hh:hh + 1], None, mybir.AluOpType.mult)

            def gather_only(tab_d, pool_g, rep):
                for b in range(NBLKS):
                    S0, S1 = int(sbh[b, 0]), int(sbh[b, 1])
                    S = S0 + S1
                    t0 = int(tile_base[b, 0])
                    G = pool_g.tile([P, S, TCOLS], dt.float32, tag="g")
                    for hh, (sh, toff) in enumerate(((S0, 0), (S1, S0))):
                        if sh == 0:
                            continue
                        s0 = (t0 + toff) * P
                        nc.gpsimd.dma_gather(
                            G[:, toff:toff + sh, :],
                            tab_d[hh * HROWS:(hh + 1) * HROWS, :],
                            idx16_t[:, s0 // 16:(s0 + sh * P) // 16],
                            sh * P, sh * P, TCOLS, single_packet=False)
                    jk = pool_g.tile([P, 1], dt.float32, tag="jk")
                    nc.vector.tensor_copy(jk[:], G[:, 0, 0:1])

            def one_layer(lhs_tile, rhs_t, er_sb, t_stg, tag, do_edge=True,
                          do_ag=True):
                slice_d = dpool.tile([RSLICE, TCOLS], dt.float32,
                                     tag=f"slice{tag}")
                tab_d = dpool.tile([TROWS, TCOLS], dt.float32,
                                   addr_space="Shared", tag=f"tab{tag}")
                with (
                    tc.tile_pool(name=f"fcps{tag}", bufs=2,
                                 space="PSUM") as fcps,
                    tc.tile_pool(name=f"fcstg{tag}", bufs=3) as fcstg,
                ):
                    fc_phase(lhs_tile, rhs_t, slice_d, er_sb, fcps, fcstg)
                if not do_ag:
                    return None
                nc.gpsimd.collective_compute(
                    "AllGather", mybir.AluOpType.bypass,
                    replica_groups=[list(range(NCORES))],
                    ins=[slice_d.opt()], outs=[tab_d.opt()])
                if not do_edge:
                    return None
                with (
                    tc.tile_pool(name=f"gpool{tag}", bufs=3) as gpool,
                    tc.tile_pool(name=f"epool{tag}", bufs=4) as epool,
                    tc.tile_pool(name=f"pspool{tag}", bufs=4,
                                 space="PSUM") as psp,
                    tc.tile_pool(name=f"epspool{tag}", bufs=3,
                                 space="PSUM") as epsp,
                ):
                    if do_edge == "gath":
                        gather_only(tab_d, gpool, tag)
                    else:
                        edge_phase(tab_d, t_stg, er_sb, gpool, epool, psp,
                                   epsp)
                return tab_d

            if variant in ("edge1", "gath1", "ag", "fc"):
                for _rep in range(repeat):
                    de = {"edge1": True, "gath1": "gath",
                          "ag": False, "fc": False}[variant]
                    one_layer(hT_t, rhs1_t, er1_sb, t1, f"1r{_rep}",
                              do_edge=de, do_ag=(variant != "fc"))
                repeat = 0  # skip the full pipeline below

            for _rep in range(repeat):
                one_layer(hT_t, rhs1_t, er1_sb, t1, f"1r{_rep}")

                for nb in range(NBLKS):
                    nc.sync.dma_start_transpose(t1T[:, nb * P:(nb + 1) * P],
                                                t1[:, nb, :])
                with (
                    tc.tile_pool(name="x2ps", bufs=2, space="PSUM") as x2ps,
                ):
                    CH = 512
                    nch = NCOLS // CH
                    rem = NCOLS - nch * CH
                    for ch in range(nch + (1 if rem else 0)):
                        w = CH if ch < nch else rem
                        ps = x2ps.tile([HID, CH], dt.float32, tag="xps")
                        nc.tensor.matmul(ps[:, 0:w], l1w_t[:],
                                         t1T[:, ch * CH:ch * CH + w],
                                         start=True, stop=True)
                        nc.scalar.activation(xT[:, ch * CH:ch * CH + w],
                                             ps[:, 0:w],
                                             mybir.ActivationFunctionType.Relu,
                                             bias=b1_t[:])

                one_layer(xT, rhs2_t, er2_sb, t2, f"2r{_rep}")

                for nb in range(NBLKS):
                    nc.sync.dma_start_transpose(t2T[:, nb * P:(nb + 1) * P],
                                                t2[:, nb, :])
                with (
                    tc.tile_pool(name="ops", bufs=2, space="PSUM") as ops,
                    tc.tile_pool(name="ostg", bufs=3) as ostg,
                ):
                    for nb in range(NBLKS):
                        ps = ops.tile([P, OUT], dt.float32, tag="ops")
                        nc.tensor.matmul(ps[:], t2T[:, nb * P:(nb + 1) * P],
                                         l2w_t[:], start=True, stop=True)
                        og = ostg.tile([P, OUT], dt.float32, tag="og")
                        nc.vector.tensor_tensor(og[:], ps[:], b2_t[:],
                                                mybir.AluOpType.add)
                        r0 = nb * P
                        r1 = min(r0 + P, NSHARD)
                        if r1 > r0:
                            nc.sync.dma_start(d_out[r0:r1, :], og[0:r1 - r0, :])

    nc.compile()
    return nc


def kernel(**inputs) -> np.ndarray:
    from concourse.bass_utils import run_bass_kernel_spmd

    args = {k: np.asarray(v) for k, v in inputs.items()}
    shared, per_core, sbh = preprocess(args)
    nc = build(sbh)
    in_maps = [{**shared, **pc} for pc in per_core]
    res = run_bass_kernel_spmd(nc, in_maps, list(range(NCORES)))
    out = np.concatenate([res.results[c]["out"] for c in range(NCORES)], axis=0)
    return np.ascontiguousarray(out.astype(np.float32))
